# revision 45
# baseline (speedup 1.0000x reference)
"""Trainium2 Bass kernel for nn_DiscreteLoss (data-parallel over batch).

Contract: kernel(**inputs) takes the FULL unsharded inputs (B=64) and
returns the FULL scalar loss.  Internally the batch dim is sharded over
8 NeuronCores (8 batches each); each core produces partial sums which
the host combines in float64.

Device-side strategy per core (8 batches):
  - all bulk tensors ship as fp8 (e4m3); the loss tolerates it easily
    (host-simulated rel err ~7e-4), halving HBM traffic vs bf16.
  - the mapping gather AND the ground-truth subtraction are fused into a
    single DoubleRow fp8 matmul per 512-column block: the 256-deep
    contraction holds [one-hot gather rows ; -I rows], so PSUM receives
    (x_gathered - gt) directly at 0.5 cycles/column.  The one-hot
    weights (an encoding of the integer `mapping` input, like the
    baseline's bf16 mapf) are host-packed with the data.
  - normalizer folds: rz/zs x4, masks x1/16 and landmark-point-columns
    x sqrt(2) are folded into the data so one accumulator covers
    auto+disk+landmark+seg; the best_* block is scaled so a single
    accumulator covers its three terms.
  - squares: ScalarE squares chunks 1-3 (2 batches per Square+accum
    instruction, straight from a 4-bank PSUM pair); VectorE handles
    chunk 0 via PSUM->bf16 cast + bf16 self-multiply + reduce (only one
    PSUM operand is allowed per DVE op; tensor_tensor_reduce and any
    GpSimd compute fault or crawl on this runtime, all HW-verified),
    plus the KL multiply/reduce and the best block.
  - the chip power-shares DMA bandwidth and engine clocks (throttle is
    visible in every profile), so the schedule minimizes TOTAL work:
    fp8 halves DMA energy, the DoubleRow gather is the only TensorE
    work, and each PSUM element is touched exactly once per engine.
"""

import contextlib
import ctypes
import os
import sys
import types

for _p in ("/opt/trn_rl_repo", "/root/.axon_site/_ro/trn_rl_repo"):
    if os.path.isdir(_p) and _p not in sys.path:
        sys.path.append(_p)

import numpy as np

# --- problem constants (hardcoded per spec) ---
B, S, N, D, V = 64, 128, 128, 512, 128
N_CORES = 8
BPC = B // N_CORES          # batches per core = 8
ALPHA, BETA, GAMMA, EPS = 1.0, 0.1, 1.0, 1e-20
MARK = (0, 29, 88, 117)
ACT_PAIRS = (0, 2, 3)    # batch pairs squared via ScalarE Square+accum
DVE_PAIR = 1             # batch pair squared via VectorE cast+mult+reduce

_CACHE = {}


def _install_ntff_hook_shim():
    """run_bass_kernel_spmd(trace=True) looks for antenv.axon_hooks, which
    this image lacks; recreate the ctypes hook against libaxon_pjrt.so."""
    if "antenv.axon_hooks" in sys.modules:
        return
    so_path = "/opt/axon/libaxon_pjrt.so"

    def _get_hook():
        if not os.path.exists(so_path):
            return None
        lib = ctypes.CDLL(so_path)
        if not hasattr(lib, "axon_start_nrt_profile"):
            return None
        lib.axon_start_nrt_profile.argtypes = [
            ctypes.POINTER(ctypes.c_int64), ctypes.c_size_t]
        lib.axon_start_nrt_profile.restype = ctypes.c_int64
        lib.axon_stop_nrt_profile.argtypes = [ctypes.c_char_p]
        lib.axon_stop_nrt_profile.restype = ctypes.c_int64

        @contextlib.contextmanager
        def _hook(output_dir, device_ids):
            import jax
            jax.devices()
            if device_ids:
                ids = (ctypes.c_int64 * len(device_ids))(*device_ids)
                rc = lib.axon_start_nrt_profile(ids, len(device_ids))
            else:
                rc = lib.axon_start_nrt_profile(None, 0)
            if rc != 0:
                raise RuntimeError(f"axon_start_nrt_profile rc={rc}")
            try:
                yield
            finally:
                n = lib.axon_stop_nrt_profile(str(output_dir).encode())
                if n < 0:
                    raise RuntimeError(f"axon_stop_nrt_profile rc={n}")

        return _hook

    mod = types.ModuleType("antenv.axon_hooks")
    mod.get_axon_ntff_profile_hook = _get_hook
    mod.set_axon_ntff_profile_hook = lambda h: None
    sys.modules["antenv.axon_hooks"] = mod


def _build_program():
    import concourse.bacc as bacc
    import concourse.tile as tile
    from concourse import mybir

    f32 = mybir.dt.float32
    bf16 = mybir.dt.bfloat16
    f8 = mybir.dt.float8e4
    nc = bacc.Bacc(None, target_bir_lowering=False, debug=False)

    # ---- per-core DRAM parameters (host pre-packed, all fp8) ----
    # data[s, b, 0:1024]  = [rzs | pts*wland | masks/16]      (gathered)
    # data[s, b, 1024:2048] = [zs | pts_gt*wland | masks_gt/16] (subtracted)
    d_data = nc.declare_dram_parameter("data", [S, BPC, 2048], f8, isOutput=False)
    # head[:, 0:16, :] = per-batch DoubleRow weights [E_b | -I]
    # head[:, 16:24, :] = qy (partition=s, per-batch 128 cols)
    # head[:, 24, :]   = best block: [brz|bp|bm] cols 0:64, [lg|bpg|bmg] 64:128
    d_head = nc.declare_dram_parameter("head", [128, 25, 128], f8, isOutput=False)
    # outputs
    d_oact = nc.declare_dram_parameter("o_act", [128, 2], f32, isOutput=True)
    d_odve = nc.declare_dram_parameter("o_dve", [128, 5], f32, isOutput=True)

    SQUARE = mybir.ActivationFunctionType.Square
    COPY = mybir.ActivationFunctionType.Copy
    LN = mybir.ActivationFunctionType.Ln
    AL = mybir.AluOpType
    AX = mybir.AxisListType
    DR = mybir.MatmulPerfMode.DoubleRow

    with tile.TileContext(nc) as tc:
        with contextlib.ExitStack() as ctx:
            singles = ctx.enter_context(tc.tile_pool(name="singles", bufs=1))
            junkp = ctx.enter_context(tc.tile_pool(name="junk", bufs=2))
            psp = ctx.enter_context(tc.tile_pool(name="ps", bufs=2, space="PSUM"))

            # ---- SBUF tiles ----
            t_head = singles.tile([128, 25, 128], f8)
            chunks = [singles.tile([128, 2, 1024], f8, tag=f"d{k}",
                                   name=f"chunk{k}")
                      for k in range(8)]
            t_lnq = singles.tile([128, 8, 128], bf16)
            t_eps = singles.tile([128, 1], f32)
            db = singles.tile([128, 64], bf16)
            a_act = singles.tile([128, 2], f32)
            a_dve = singles.tile([128, 5], f32)

            # ---- pin the act table once: natural_log covers ln+square ----
            from concourse.hw_specs import get_activation_tables
            tabs = list(get_activation_tables(nc.m.arch).items())
            set_id = next(i for i, (_, funcs) in enumerate(tabs)
                          if LN in funcs and SQUARE in funcs)
            nc.scalar.add_instruction(mybir.InstLoadActFuncSet(
                act_func_set_id=set_id,
                name=nc.get_next_instruction_name(), ins=[], outs=[]))

            # ---- input DMAs: head + 8 single-batch chunks; batch 0 rides
            # the scalar ring so it lands first and the pipeline starts
            # early, everything else queues on the sync ring ----
            nc.scalar.dma_start(out=chunks[0][:], in_=d_data.ap()[:, 0:1, :])
            nc.sync.dma_start(out=t_head[:], in_=d_head.ap())
            for b in range(1, 8):
                nc.sync.dma_start(out=chunks[b][:],
                                  in_=d_data.ap()[:, b:b + 1, :])

            nc.vector.memset(t_eps[:], EPS)

            # ---- gather+subtract matmuls: one DoubleRow pair per batch;
            # each batch pair shares a 4-bank PSUM pair tile ----
            pps = []
            for k in range(4):
                pp = psp.tile([128, 2, 1024], f32, tag="pp", name=f"pp{k}")
                pps.append(pp)
                for j in range(2):
                    b = 2 * k + j
                    wb = t_head[:, 2 * b:2 * b + 2, :]      # [128, 2, 128]
                    rhs = chunks[b]                         # [128, 2, 1024]
                    nc.tensor.matmul(pp[:, j, 0:512], lhsT=wb,
                                     rhs=rhs[:, :, 0:512],
                                     start=True, stop=True, perf_mode=DR)
                    nc.tensor.matmul(pp[:, j, 512:1024], lhsT=wb,
                                     rhs=rhs[:, :, 512:1024],
                                     start=True, stop=True, perf_mode=DR)

            # ---- ScalarE: Ln first (its table set also covers Square),
            # then five Square batches into fp8 jsq tiles ----
            qy_ap = t_head[:, 16:24, :]
            nc.scalar.activation(out=t_lnq[:], in_=qy_ap, func=LN,
                                 scale=float(V), bias=t_eps[:])
            for i, k in enumerate(ACT_PAIRS):
                jq = junkp.tile([128, 2048], bf16, tag="jq", name=f"jq{k}")
                # the LAST pair's accum joins the o_dve store so the
                # final store does not serialize behind a second one
                acc = (a_act[:, i:i + 1] if i < 2 else a_dve[:, 4:5])
                nc.scalar.activation(out=jq[:], in_=pps[k][:], func=SQUARE,
                                     accum_out=acc)

            # ---- VectorE: best-block diff+square+reduce, PSUM->bf16 casts
            # and bf16 squares for its batches ----
            nc.vector.tensor_sub(db[:], t_head[:, 24, 0:64],
                                 t_head[:, 24, 64:128])
            jb = junkp.tile([128, 64], bf16, tag="jb")
            nc.vector.tensor_tensor(out=jb[:], in0=db[:], in1=db[:],
                                    op=AL.mult)
            nc.vector.tensor_reduce(out=a_dve[:, 3:4], in_=jb[:],
                                    axis=AX.X, op=AL.add)
            # casts first: they are what holds the PSUM pair slot, so
            # they must win scheduler priority over the SBUF-side ops
            jcs = []
            for j in range(2):
                jc = junkp.tile([128, 1024], bf16, tag="jc", name=f"jc{j}")
                jcs.append(jc)
                nc.vector.tensor_copy(out=jc[:], in_=pps[DVE_PAIR][:, j, :])
            for j in range(2):
                jd = junkp.tile([128, 1024], bf16, tag="jd", name=f"jd{j}")
                nc.vector.tensor_tensor(out=jd[:], in0=jcs[j], in1=jcs[j],
                                        op=AL.mult)
                nc.vector.tensor_reduce(out=a_dve[:, j:j + 1], in_=jd[:],
                                        axis=AX.X, op=AL.add)

            # ---- KL product q * ln(128 q) + reduce on DVE ----
            jk = junkp.tile([128, 1024], bf16, tag="jk")
            nc.vector.tensor_tensor(out=jk[:], in0=qy_ap, in1=t_lnq[:],
                                    op=AL.mult)
            nc.vector.tensor_reduce(out=a_dve[:, 2:3], in_=jk[:],
                                    axis=AX.X, op=AL.add)

            # ---- store partials: the first two pair accums go out as
            # soon as they exist; the single final store carries the rest ----
            nc.sync.dma_start(out=d_oact.ap(), in_=a_act[:])
            # the scalar engine writes the last accumulator itself, so the
            # final store issues in-order with no cross-engine sem hop
            nc.scalar.dma_start(out=d_odve.ap(), in_=a_dve[:])

    nc.compile()
    return nc


def _get_program():
    if "nc" not in _CACHE:
        _CACHE["nc"] = _build_program()
    return _CACHE["nc"]


def _shard_inputs(inputs):
    """Pack the full B=64 inputs into 8 per-core fp8 input maps."""
    import ml_dtypes
    f8 = ml_dtypes.float8_e4m3
    f = lambda k: np.asarray(inputs[k], dtype=np.float32)

    wland = np.ones(N, dtype=np.float32)
    for n in MARK:
        wland[n] = np.sqrt(np.float32(2.0))

    # gathered block  [B,S,1024] and subtracted block [B,S,1024]
    pts = f("pts") * wland[None, None, :, None]
    ptsg = f("pts_gt") * wland[None, None, :, None]
    xg = np.concatenate([f("rzs") * np.float32(4.0),
                         pts.reshape(B, S, 2 * N),
                         f("masks").reshape(B, S, 2 * N) * np.float32(1 / 16)],
                        axis=2)
    gt = np.concatenate([f("zs") * np.float32(4.0),
                         ptsg.reshape(B, S, 2 * N),
                         f("masks_gt").reshape(B, S, 2 * N) * np.float32(1 / 16)],
                        axis=2)
    data = np.concatenate([xg, gt], axis=2).astype(f8)      # [B,S,2048]

    mapping = np.asarray(inputs["mapping"]).astype(np.int64)  # [B,S]
    iota = np.arange(128, dtype=np.int64)
    # one-hot E_b[p, m] = (mapping[b, m] == p), negI[p, m] = -(p == m)
    onehot = (mapping[:, None, :] == iota[None, :, None]).astype(f8)  # [B,128,128]
    negi = (-np.eye(128, dtype=np.float32)).astype(f8)

    qy = f("qy")

    # best block, scaled so one accumulator (div B*D) covers all 3 terms:
    #   auto: x1; pt: *wbest/64 (folds best_landmark and the /(2 B^2 N^2));
    #   seg: slice-zeroed, x2 (folds /(128 B)).
    wbest = np.ones(N, dtype=np.float32)
    rb = np.sqrt(np.float32(1.0 + 2.0 * B * N))
    for n in MARK:
        wbest[n] = rb
    bp = f("best_pt") * wbest[None, :, None] * np.float32(1 / 64)
    bpg = f("best_pt_gt") * wbest[None, :, None] * np.float32(1 / 64)
    bm = f("best_mask").copy() * np.float32(2.0)
    bmg = f("best_mask_gt").copy() * np.float32(2.0)
    bm[:, :32] = 0.0
    bm[:, 96:] = 0.0
    bmg[:, :32] = 0.0
    bmg[:, 96:] = 0.0
    brz = f("best_rz")
    lg = f("logits")

    in_maps = []
    for c in range(N_CORES):
        lo, hi = c * BPC, (c + 1) * BPC
        head = np.zeros((128, 25, 128), dtype=f8)
        head[:, 0:16:2, :] = onehot[lo:hi].transpose(1, 0, 2)
        head[:, 1:16:2, :] = negi[:, None, :]
        head[:, 16:24, :] = qy[lo:hi].transpose(1, 0, 2).astype(f8)
        half0 = np.concatenate([brz[lo:hi].reshape(128, 32),
                                bp[lo:hi].reshape(128, 16),
                                bm[lo:hi].reshape(128, 16)], axis=1)
        half1 = np.concatenate([lg[lo:hi].reshape(128, 32),
                                bpg[lo:hi].reshape(128, 16),
                                bmg[lo:hi].reshape(128, 16)], axis=1)
        head[:, 24, 0:64] = half0.astype(f8)
        head[:, 24, 64:128] = half1.astype(f8)
        m = {
            "data": np.ascontiguousarray(data[lo:hi].transpose(1, 0, 2)),
            "head": head,
        }
        in_maps.append(m)
    return in_maps


def _combine(results, ln_v):
    """Host-side float64 reduction of the per-core partial sums."""
    s_main = s_kld = s_best = 0.0
    for r in results:
        od = r["o_dve"].astype(np.float64)
        s_main += (r["o_act"].astype(np.float64).sum() + od[:, 0:2].sum()
                   + od[:, 4].sum())
        s_kld += od[:, 2].sum()
        s_best += od[:, 3].sum()
    # device computed sum q*ln(128 q); only valid for vector_dims == 128
    if abs(ln_v - np.log(128.0)) > 1e-12:
        raise ValueError("kernel compiled for vector_dims == 128")

    main = s_main / (B * S)
    kld = s_kld / (B * S)
    best = s_best / (B * D)
    ret = best + main + BETA * kld
    return np.float32(ret * B)


def run_sharded(inputs, trace=False):
    """Compile (cached), run on the 8 cores, return (scalar, BassKernelResults)."""
    _install_ntff_hook_shim()
    from concourse.bass_utils import run_bass_kernel_spmd

    ln_v = float(np.log(float(inputs["vector_dims"])))
    nc = _get_program()
    in_maps = _shard_inputs(inputs)
    res = run_bass_kernel_spmd(nc, in_maps, list(range(N_CORES)), trace=trace)
    return _combine(res.results, ln_v), res


def kernel(**inputs) -> np.ndarray:
    out, _ = run_sharded(inputs, trace=False)
    return out


# revision 46
# speedup vs baseline: 1.0190x; 1.0190x over previous
"""Trainium2 Bass kernel for nn_DiscreteLoss (data-parallel over batch).

Contract: kernel(**inputs) takes the FULL unsharded inputs (B=64) and
returns the FULL scalar loss.  Internally the batch dim is sharded over
8 NeuronCores (8 batches each); each core produces partial sums which
the host combines in float64.

Device-side strategy per core (8 batches):
  - all bulk tensors ship as fp8 (e4m3); the loss tolerates it easily
    (host-simulated rel err ~7e-4), halving HBM traffic vs bf16.
  - the mapping gather AND the ground-truth subtraction are fused into a
    single DoubleRow fp8 matmul per 512-column block: the 256-deep
    contraction holds [one-hot gather rows ; -I rows], so PSUM receives
    (x_gathered - gt) directly at 0.5 cycles/column.  The one-hot
    weights (an encoding of the integer `mapping` input, like the
    baseline's bf16 mapf) are host-packed with the data.
  - normalizer folds: rz/zs x4, masks x1/16 and landmark-point-columns
    x sqrt(2) are folded into the data so one accumulator covers
    auto+disk+landmark+seg; the best_* block is scaled so a single
    accumulator covers its three terms.
  - squares: ScalarE squares chunks 1-3 (2 batches per Square+accum
    instruction, straight from a 4-bank PSUM pair); VectorE handles
    chunk 0 via PSUM->bf16 cast + bf16 self-multiply + reduce (only one
    PSUM operand is allowed per DVE op; tensor_tensor_reduce and any
    GpSimd compute fault or crawl on this runtime, all HW-verified),
    plus the KL multiply/reduce and the best block.
  - the chip power-shares DMA bandwidth and engine clocks (throttle is
    visible in every profile), so the schedule minimizes TOTAL work:
    fp8 halves DMA energy, the DoubleRow gather is the only TensorE
    work, and each PSUM element is touched exactly once per engine.
"""

import contextlib
import ctypes
import os
import sys
import types

for _p in ("/opt/trn_rl_repo", "/root/.axon_site/_ro/trn_rl_repo"):
    if os.path.isdir(_p) and _p not in sys.path:
        sys.path.append(_p)

import numpy as np

# --- problem constants (hardcoded per spec) ---
B, S, N, D, V = 64, 128, 128, 512, 128
N_CORES = 8
BPC = B // N_CORES          # batches per core = 8
ALPHA, BETA, GAMMA, EPS = 1.0, 0.1, 1.0, 1e-20
MARK = (0, 29, 88, 117)
ACT_PAIRS = (0, 2, 3)    # batch pairs squared via ScalarE Square+accum
DVE_PAIR = 1             # batch pair squared via VectorE cast+mult+reduce

_CACHE = {}


def _install_ntff_hook_shim():
    """run_bass_kernel_spmd(trace=True) looks for antenv.axon_hooks, which
    this image lacks; recreate the ctypes hook against libaxon_pjrt.so."""
    if "antenv.axon_hooks" in sys.modules:
        return
    so_path = "/opt/axon/libaxon_pjrt.so"

    def _get_hook():
        if not os.path.exists(so_path):
            return None
        lib = ctypes.CDLL(so_path)
        if not hasattr(lib, "axon_start_nrt_profile"):
            return None
        lib.axon_start_nrt_profile.argtypes = [
            ctypes.POINTER(ctypes.c_int64), ctypes.c_size_t]
        lib.axon_start_nrt_profile.restype = ctypes.c_int64
        lib.axon_stop_nrt_profile.argtypes = [ctypes.c_char_p]
        lib.axon_stop_nrt_profile.restype = ctypes.c_int64

        @contextlib.contextmanager
        def _hook(output_dir, device_ids):
            import jax
            jax.devices()
            if device_ids:
                ids = (ctypes.c_int64 * len(device_ids))(*device_ids)
                rc = lib.axon_start_nrt_profile(ids, len(device_ids))
            else:
                rc = lib.axon_start_nrt_profile(None, 0)
            if rc != 0:
                raise RuntimeError(f"axon_start_nrt_profile rc={rc}")
            try:
                yield
            finally:
                n = lib.axon_stop_nrt_profile(str(output_dir).encode())
                if n < 0:
                    raise RuntimeError(f"axon_stop_nrt_profile rc={n}")

        return _hook

    mod = types.ModuleType("antenv.axon_hooks")
    mod.get_axon_ntff_profile_hook = _get_hook
    mod.set_axon_ntff_profile_hook = lambda h: None
    sys.modules["antenv.axon_hooks"] = mod


def _build_program():
    import concourse.bacc as bacc
    import concourse.tile as tile
    from concourse import mybir

    f32 = mybir.dt.float32
    bf16 = mybir.dt.bfloat16
    f8 = mybir.dt.float8e4
    nc = bacc.Bacc(None, target_bir_lowering=False, debug=False)

    # ---- per-core DRAM parameters (host pre-packed, all fp8) ----
    # data[s, b, 0:1024]  = [rzs | pts*wland | masks/16]      (gathered)
    # data[s, b, 1024:2048] = [zs | pts_gt*wland | masks_gt/16] (subtracted)
    d_data = nc.declare_dram_parameter("data", [S, BPC, 2048], f8, isOutput=False)
    # head[:, 0:16, :] = per-batch DoubleRow weights [E_b | -I]
    # head[:, 16:24, :] = qy (partition=s, per-batch 128 cols)
    # head[:, 24, :]   = best block: [brz|bp|bm] cols 0:64, [lg|bpg|bmg] 64:128
    d_head = nc.declare_dram_parameter("head", [128, 25, 128], f8, isOutput=False)
    # outputs
    d_oact = nc.declare_dram_parameter("o_act", [128, 2], f32, isOutput=True)
    d_odve = nc.declare_dram_parameter("o_dve", [128, 5], f32, isOutput=True)

    SQUARE = mybir.ActivationFunctionType.Square
    COPY = mybir.ActivationFunctionType.Copy
    LN = mybir.ActivationFunctionType.Ln
    AL = mybir.AluOpType
    AX = mybir.AxisListType
    DR = mybir.MatmulPerfMode.DoubleRow

    with tile.TileContext(nc) as tc:
        with contextlib.ExitStack() as ctx:
            singles = ctx.enter_context(tc.tile_pool(name="singles", bufs=1))
            junkp = ctx.enter_context(tc.tile_pool(name="junk", bufs=2))
            psp = ctx.enter_context(tc.tile_pool(name="ps", bufs=2, space="PSUM"))

            # ---- SBUF tiles ----
            t_head = singles.tile([128, 25, 128], f8)
            chunks = [singles.tile([128, 2, 1024], f8, tag=f"d{k}",
                                   name=f"chunk{k}")
                      for k in range(8)]
            t_lnq = singles.tile([128, 8, 128], bf16)
            t_eps = singles.tile([128, 1], f32)
            db = singles.tile([128, 64], bf16)
            a_act = singles.tile([128, 2], f32)
            a_dve = singles.tile([128, 5], f32)

            # ---- pin the act table once: natural_log covers ln+square ----
            from concourse.hw_specs import get_activation_tables
            tabs = list(get_activation_tables(nc.m.arch).items())
            set_id = next(i for i, (_, funcs) in enumerate(tabs)
                          if LN in funcs and SQUARE in funcs)
            nc.scalar.add_instruction(mybir.InstLoadActFuncSet(
                act_func_set_id=set_id,
                name=nc.get_next_instruction_name(), ins=[], outs=[]))

            # ---- input DMAs: head + 8 single-batch chunks; batch 0 rides
            # the scalar ring so it lands first and the pipeline starts
            # early, everything else queues on the sync ring ----
            nc.scalar.dma_start(out=chunks[0][:], in_=d_data.ap()[:, 0:1, :])
            # W first (it gates every matmul); qy/best follow chunk1 --
            # the Ln has slack, the matmul pipeline does not
            nc.sync.dma_start(out=t_head[:, 0:16, :],
                              in_=d_head.ap()[:, 0:16, :])
            nc.sync.dma_start(out=chunks[1][:], in_=d_data.ap()[:, 1:2, :])
            nc.sync.dma_start(out=t_head[:, 16:25, :],
                              in_=d_head.ap()[:, 16:25, :])
            for b in range(2, 8):
                nc.sync.dma_start(out=chunks[b][:],
                                  in_=d_data.ap()[:, b:b + 1, :])

            nc.vector.memset(t_eps[:], EPS)

            # ---- gather+subtract matmuls: one DoubleRow pair per batch;
            # each batch pair shares a 4-bank PSUM pair tile ----
            pps = []
            for k in range(4):
                pp = psp.tile([128, 2, 1024], f32, tag="pp", name=f"pp{k}")
                pps.append(pp)
                for j in range(2):
                    b = 2 * k + j
                    wb = t_head[:, 2 * b:2 * b + 2, :]      # [128, 2, 128]
                    rhs = chunks[b]                         # [128, 2, 1024]
                    nc.tensor.matmul(pp[:, j, 0:512], lhsT=wb,
                                     rhs=rhs[:, :, 0:512],
                                     start=True, stop=True, perf_mode=DR)
                    nc.tensor.matmul(pp[:, j, 512:1024], lhsT=wb,
                                     rhs=rhs[:, :, 512:1024],
                                     start=True, stop=True, perf_mode=DR)

            # ---- ScalarE: Ln first (its table set also covers Square),
            # then five Square batches into fp8 jsq tiles ----
            qy_ap = t_head[:, 16:24, :]
            nc.scalar.activation(out=t_lnq[:], in_=qy_ap, func=LN,
                                 scale=float(V), bias=t_eps[:])
            for i, k in enumerate(ACT_PAIRS):
                jq = junkp.tile([128, 2048], bf16, tag="jq", name=f"jq{k}")
                # the LAST pair's accum joins the o_dve store so the
                # final store does not serialize behind a second one
                acc = (a_act[:, i:i + 1] if i < 2 else a_dve[:, 4:5])
                nc.scalar.activation(out=jq[:], in_=pps[k][:], func=SQUARE,
                                     accum_out=acc)

            # ---- VectorE: best-block diff+square+reduce, PSUM->bf16 casts
            # and bf16 squares for its batches ----
            nc.vector.tensor_sub(db[:], t_head[:, 24, 0:64],
                                 t_head[:, 24, 64:128])
            jb = junkp.tile([128, 64], bf16, tag="jb")
            nc.vector.tensor_tensor(out=jb[:], in0=db[:], in1=db[:],
                                    op=AL.mult)
            nc.vector.tensor_reduce(out=a_dve[:, 3:4], in_=jb[:],
                                    axis=AX.X, op=AL.add)
            # casts first: they are what holds the PSUM pair slot, so
            # they must win scheduler priority over the SBUF-side ops
            jcs = []
            for j in range(2):
                jc = junkp.tile([128, 1024], bf16, tag="jc", name=f"jc{j}")
                jcs.append(jc)
                nc.vector.tensor_copy(out=jc[:], in_=pps[DVE_PAIR][:, j, :])
            for j in range(2):
                jd = junkp.tile([128, 1024], bf16, tag="jd", name=f"jd{j}")
                nc.vector.tensor_tensor(out=jd[:], in0=jcs[j], in1=jcs[j],
                                        op=AL.mult)
                nc.vector.tensor_reduce(out=a_dve[:, j:j + 1], in_=jd[:],
                                        axis=AX.X, op=AL.add)

            # ---- KL product q * ln(128 q) + reduce on DVE ----
            jk = junkp.tile([128, 1024], bf16, tag="jk")
            nc.vector.tensor_tensor(out=jk[:], in0=qy_ap, in1=t_lnq[:],
                                    op=AL.mult)
            nc.vector.tensor_reduce(out=a_dve[:, 2:3], in_=jk[:],
                                    axis=AX.X, op=AL.add)

            # ---- store partials: the first two pair accums go out as
            # soon as they exist; the single final store carries the rest ----
            nc.sync.dma_start(out=d_oact.ap(), in_=a_act[:])
            # the scalar engine writes the last accumulator itself, so the
            # final store issues in-order with no cross-engine sem hop
            nc.scalar.dma_start(out=d_odve.ap(), in_=a_dve[:])

    nc.compile()
    return nc


def _get_program():
    if "nc" not in _CACHE:
        _CACHE["nc"] = _build_program()
    return _CACHE["nc"]


def _shard_inputs(inputs):
    """Pack the full B=64 inputs into 8 per-core fp8 input maps."""
    import ml_dtypes
    f8 = ml_dtypes.float8_e4m3
    f = lambda k: np.asarray(inputs[k], dtype=np.float32)

    wland = np.ones(N, dtype=np.float32)
    for n in MARK:
        wland[n] = np.sqrt(np.float32(2.0))

    # gathered block  [B,S,1024] and subtracted block [B,S,1024]
    pts = f("pts") * wland[None, None, :, None]
    ptsg = f("pts_gt") * wland[None, None, :, None]
    xg = np.concatenate([f("rzs") * np.float32(4.0),
                         pts.reshape(B, S, 2 * N),
                         f("masks").reshape(B, S, 2 * N) * np.float32(1 / 16)],
                        axis=2)
    gt = np.concatenate([f("zs") * np.float32(4.0),
                         ptsg.reshape(B, S, 2 * N),
                         f("masks_gt").reshape(B, S, 2 * N) * np.float32(1 / 16)],
                        axis=2)
    data = np.concatenate([xg, gt], axis=2).astype(f8)      # [B,S,2048]

    mapping = np.asarray(inputs["mapping"]).astype(np.int64)  # [B,S]
    iota = np.arange(128, dtype=np.int64)
    # one-hot E_b[p, m] = (mapping[b, m] == p), negI[p, m] = -(p == m)
    onehot = (mapping[:, None, :] == iota[None, :, None]).astype(f8)  # [B,128,128]
    negi = (-np.eye(128, dtype=np.float32)).astype(f8)

    qy = f("qy")

    # best block, scaled so one accumulator (div B*D) covers all 3 terms:
    #   auto: x1; pt: *wbest/64 (folds best_landmark and the /(2 B^2 N^2));
    #   seg: slice-zeroed, x2 (folds /(128 B)).
    wbest = np.ones(N, dtype=np.float32)
    rb = np.sqrt(np.float32(1.0 + 2.0 * B * N))
    for n in MARK:
        wbest[n] = rb
    bp = f("best_pt") * wbest[None, :, None] * np.float32(1 / 64)
    bpg = f("best_pt_gt") * wbest[None, :, None] * np.float32(1 / 64)
    bm = f("best_mask").copy() * np.float32(2.0)
    bmg = f("best_mask_gt").copy() * np.float32(2.0)
    bm[:, :32] = 0.0
    bm[:, 96:] = 0.0
    bmg[:, :32] = 0.0
    bmg[:, 96:] = 0.0
    brz = f("best_rz")
    lg = f("logits")

    in_maps = []
    for c in range(N_CORES):
        lo, hi = c * BPC, (c + 1) * BPC
        head = np.zeros((128, 25, 128), dtype=f8)
        head[:, 0:16:2, :] = onehot[lo:hi].transpose(1, 0, 2)
        head[:, 1:16:2, :] = negi[:, None, :]
        head[:, 16:24, :] = qy[lo:hi].transpose(1, 0, 2).astype(f8)
        half0 = np.concatenate([brz[lo:hi].reshape(128, 32),
                                bp[lo:hi].reshape(128, 16),
                                bm[lo:hi].reshape(128, 16)], axis=1)
        half1 = np.concatenate([lg[lo:hi].reshape(128, 32),
                                bpg[lo:hi].reshape(128, 16),
                                bmg[lo:hi].reshape(128, 16)], axis=1)
        head[:, 24, 0:64] = half0.astype(f8)
        head[:, 24, 64:128] = half1.astype(f8)
        m = {
            "data": np.ascontiguousarray(data[lo:hi].transpose(1, 0, 2)),
            "head": head,
        }
        in_maps.append(m)
    return in_maps


def _combine(results, ln_v):
    """Host-side float64 reduction of the per-core partial sums."""
    s_main = s_kld = s_best = 0.0
    for r in results:
        od = r["o_dve"].astype(np.float64)
        s_main += (r["o_act"].astype(np.float64).sum() + od[:, 0:2].sum()
                   + od[:, 4].sum())
        s_kld += od[:, 2].sum()
        s_best += od[:, 3].sum()
    # device computed sum q*ln(128 q); only valid for vector_dims == 128
    if abs(ln_v - np.log(128.0)) > 1e-12:
        raise ValueError("kernel compiled for vector_dims == 128")

    main = s_main / (B * S)
    kld = s_kld / (B * S)
    best = s_best / (B * D)
    ret = best + main + BETA * kld
    return np.float32(ret * B)


def run_sharded(inputs, trace=False):
    """Compile (cached), run on the 8 cores, return (scalar, BassKernelResults)."""
    _install_ntff_hook_shim()
    from concourse.bass_utils import run_bass_kernel_spmd

    ln_v = float(np.log(float(inputs["vector_dims"])))
    nc = _get_program()
    in_maps = _shard_inputs(inputs)
    res = run_bass_kernel_spmd(nc, in_maps, list(range(N_CORES)), trace=trace)
    return _combine(res.results, ln_v), res


def kernel(**inputs) -> np.ndarray:
    out, _ = run_sharded(inputs, trace=False)
    return out
